# revision 32
# baseline (speedup 1.0000x reference)
"""Trainium2 Bass kernel for DWConvBlock3D:
depthwise 3x3x3 conv (pad 1) + InstanceNorm3d + ReLU on x:(2,64,64,128,128) f32.

Strategy (8 NeuronCores, channel sharding => zero communication):
  - Each core owns 8 channels x 2 batches = 16 (b,c) "pairs".
  - Layout per pair: H=128 on SBUF partitions, (D,W) on the free dim.
  - The conv runs on TensorE in fp16: for each (kd,kw) of the 9 off-H taps, a
    128x128 banded matrix (3 diagonals = the kh taps, built host-side from w)
    multiplies a (d,w)-shifted view of the x tile; the 9 matmuls accumulate in
    fp32 PSUM.  H zero-padding falls out of the band structure; D/W edges are
    handled by clipping the shifted matmuls (PSUM has_written gives correct
    first-write-overwrite semantics; the center tap goes first with start=True
    so the whole bank is initialized).
  - InstanceNorm stats (fp32): sum(y) comes free from the PSUM->SBUF eviction
    (ScalarE activation-copy accum_out); sum(y^2) from one VectorE
    scalar_tensor_tensor pass; cross-partition reduction via GpSimd
    partition_all_reduce (keeps TensorE's queue free of tiny matmuls).
  - Final (y*scale+bias, ReLU) is a single in-place ScalarE activation with
    per-partition scale/bias columns (already replicated by the all-reduce).
"""

import sys

if "/opt/trn_rl_repo" not in sys.path:
    sys.path.insert(0, "/opt/trn_rl_repo")

import numpy as np

B, C, D, H, W = 2, 64, 64, 128, 128
N_CORES = 8
CH_PER_CORE = C // N_CORES  # 8
N_PAIRS = B * CH_PER_CORE  # 16
WP = W + 2  # host-padded W (zero borders) -> contiguous DMA, free w-shifts
FREE = D * W  # 8192 free elements per partition per pair
NV = D * H * W  # normalization element count per (b,c)
EPS = 1e-5
CHUNK_D = 4  # d-slices per PSUM bank chunk (4*128 = 512 fp32 = 1 bank)
# center tap first: start=True covers the full bank (edge taps are clipped)
TAP_ORDER = [(1, 1), (0, 0), (0, 1), (0, 2), (1, 0), (1, 2), (2, 0), (2, 1), (2, 2)]


def build_program(d=D, n_pairs=N_PAIRS, ch_per_core=CH_PER_CORE):
    import concourse.bacc as bacc
    import concourse.mybir as mybir
    from concourse import bass_isa
    from concourse.tile import TileContext

    free = d * W
    nv = d * H * W
    n_chunks = d // CHUNK_D
    # groups of 4 chunks: two groups ping-pong across the 8 PSUM banks, so
    # evictions of group g overlap the matmuls of group g+1 (no PE stall)
    groups = []
    left = n_chunks
    while left > 0:
        g = min(4, left)
        groups.append(g)
        left -= g

    f32 = mybir.dt.float32
    f16 = mybir.dt.float16
    nc = bacc.Bacc("TRN2", target_bir_lowering=False, debug=False, num_devices=N_CORES)

    xs = nc.dram_tensor("xs", [n_pairs, H, d, WP], f16, kind="ExternalInput")
    # [ci, h_in, kd, kw, h_out]: contiguous per channel so the per-channel DMA
    # is 128 fat descriptors instead of 1152 strided 256B ones
    bands = nc.dram_tensor(
        "bands", [ch_per_core, H, 3, 3, H], f16, kind="ExternalInput"
    )
    # startup blob: channel-0 band + the first d-slices of pair 0, packed so
    # ONE 128-descriptor DMA gates the first matmul
    d_head = CHUNK_D * 2 + 2
    boot = nc.dram_tensor("boot", [H, 9 * H + d_head * WP], f16, kind="ExternalInput")
    gb = nc.dram_tensor("gb", [128, 2 * n_pairs], f32, kind="ExternalInput")
    out = nc.dram_tensor("out", [n_pairs, H, free], f16, kind="ExternalOutput")

    with TileContext(nc) as tc:
        with (
            tc.tile_pool(name="singles", bufs=1) as singles,
            tc.tile_pool(name="xp", bufs=3) as xpool,
            tc.tile_pool(name="yp", bufs=3) as ypool,
            tc.tile_pool(name="st", bufs=3) as stats,
            tc.tile_pool(name="psmm", bufs=8, space="PSUM") as psum_mm,
        ):
            band_sb = singles.tile([H, ch_per_core, 3, 3, H], f16)
            gb_sb = singles.tile([128, 2 * n_pairs], f32)
            boot_sb = singles.tile([H, 9 * H + d_head * WP], f16)
            nc.sync.dma_start(out=boot_sb[:], in_=boot[:])
            nc.sync.dma_start(out=gb_sb[:], in_=gb[:])
            epsc = singles.tile([128, 1], f32)
            nc.vector.memset(epsc[:], EPS)
            # pair 0 reads channel-0's band straight out of the boot blob; the
            # band_sb copy is only needed from pair 8 (second batch) on
            x0 = boot_sb[:, 9 * H :].rearrange("p (d w) -> p d w", w=WP)

            def finish_pair(dd):
                # deferred stat-finish + normalize of pair p-1, issued in the
                # MIDDLE of pair p's conv (software pipeline): every input sem
                # has long fired by then, so the sqrt/finals never sit blocked
                # at the ScalarE FIFO head in front of upcoming evictions
                p, sm, sb2, y, o16 = dd["p"], dd["sm"], dd["sb2"], dd["y"], dd["o16"]
                mean, vpe = sm[:, 0:1], sm[:, 3:4]
                std, r0, t1 = sm[:, 4:5], sm[:, 5:6], sm[:, 6:7]
                nc.scalar.activation(
                    std, vpe, mybir.ActivationFunctionType.Sqrt, bias=epsc[:]
                )
                # DVE reciprocal is accurate to ~1e-3 or better: plenty for the
                # 2e-2 gate, so no Newton refinement
                nc.vector.reciprocal(r0, std)
                sc, bi = sb2[:, 0:1], sb2[:, 1:2]
                # scale = gamma * rstd ; bias = beta - mean*scale
                nc.vector.tensor_mul(sc, r0, gb_sb[:, p : p + 1])
                nc.vector.tensor_mul(t1, mean, sc)
                nc.vector.tensor_sub(bi, gb_sb[:, n_pairs + p : n_pairs + p + 1], t1)
                # fused normalize + ReLU -> f16 staging (pair p's dead x tile),
                # then store; split so ScalarE overlaps the out-DMA
                if p == n_pairs - 1:
                    # tail: balance the normalize across ScalarE (11/16, three
                    # slices) and DVE (5/16, two slices: mult-add then relu via
                    # max) so the last pair drains ~4.5us faster; out-DMA on
                    # the now-idle sync queue
                    cw = free // 16
                    bounds = [0, 4 * cw, 8 * cw, 12 * cw, 16 * cw]
                    for s in range(4):
                        lo, hi = bounds[s], bounds[s + 1]
                        osl = o16[:, lo:hi]
                        if s < 2:
                            nc.scalar.activation(
                                out=osl, in_=y[:, lo:hi],
                                func=mybir.ActivationFunctionType.Relu,
                                scale=sc, bias=bi,
                            )
                        else:
                            nc.vector.tensor_scalar(
                                osl, y[:, lo:hi], sc, bi,
                                op0=mybir.AluOpType.mult,
                                op1=mybir.AluOpType.add,
                            )
                            nc.vector.tensor_scalar_max(osl, osl, 0.0)
                        nc.sync.dma_start(out=out[p][:, lo:hi], in_=osl)
                else:
                    sf = free // 2
                    for s in range(2):
                        nc.scalar.activation(
                            out=o16[:, s * sf : (s + 1) * sf],
                            in_=y[:, s * sf : (s + 1) * sf],
                            func=mybir.ActivationFunctionType.Relu,
                            scale=sc,
                            bias=bi,
                        )
                        nc.gpsimd.dma_start(
                            out=out[p][:, s * sf : (s + 1) * sf],
                            in_=o16[:, s * sf : (s + 1) * sf],
                        )

            pending = None
            for p in range(n_pairs):
                ci = p % ch_per_core

                xt = xpool.tile([H, d, WP], f16, tag="xt")
                nc.sync.dma_start(out=xt[:], in_=xs[p])
                if p < ch_per_core:
                    # just-in-time per-channel band load (keeps startup short;
                    # ci=0's copy is only read from pair 8 on)
                    nc.sync.dma_start(out=band_sb[:, ci], in_=bands[ci])

                y = ypool.tile([H, free], f16, tag="y")
                sums = stats.tile([128, n_chunks], f32, tag="sums")
                sums2 = stats.tile([128, n_chunks], f32, tag="sums2")
                st2 = stats.tile([128, 2], f32, tag="st2")

                # ---- depthwise conv: 9 banded matmuls per chunk, PSUM-accumulated
                # pair 0 leads with two 2-chunk groups so its first matmuls only
                # need the small x0 head tile
                pgroups = [2, 2] + groups[1:] if p == 0 else groups
                chunk0 = 0
                for gi, gsize in enumerate(pgroups):
                    chunks = range(chunk0, chunk0 + gsize)
                    chunk0 += gsize
                    xsrc = x0 if (p == 0 and gi == 0) else xt
                    ps = {
                        c: psum_mm.tile(
                            [128, CHUNK_D, W], f32, tag="mm", name=f"mm_{p}_{c}"
                        )
                        for c in chunks
                    }
                    for t9, (kd, kw) in enumerate(TAP_ORDER):
                        if p == 0:
                            k9 = kd * 3 + kw
                            lhsT = boot_sb[:, k9 * H : (k9 + 1) * H]
                        else:
                            lhsT = band_sb[:, ci, kd, kw, :]
                        for c in chunks:
                            d0 = c * CHUNK_D
                            lo_d = max(0, d0 + kd - 1)
                            hi_d = min(d, d0 + CHUNK_D + kd - 1)
                            od = lo_d - (d0 + kd - 1)
                            nd = hi_d - lo_d
                            nc.tensor.matmul(
                                ps[c][:, od : od + nd, :],
                                lhsT,
                                xsrc[:, lo_d:hi_d, kw : kw + W],
                                start=(t9 == 0),
                                stop=(t9 == 8),
                                skip_group_check=True,
                            )
                    # ---- evict PSUM -> y (ScalarE, sum via accum_out); per-chunk
                    # sumsq on DVE right behind it, so the stats are ready at
                    # pair end (keeps finals from clogging the ScalarE FIFO
                    # ahead of the next pair's evictions)
                    for c in chunks:
                        ysl = y[:, c * CHUNK_D * W : (c + 1) * CHUNK_D * W]
                        nc.scalar.activation(
                            out=ysl,
                            in_=ps[c][:],
                            func=mybir.ActivationFunctionType.Copy,
                            accum_out=sums[:, c : c + 1],
                        )
                        sqs = stats.tile([128, CHUNK_D * W], f16, tag="sqs")
                        nc.vector.scalar_tensor_tensor(
                            out=sqs[:], in0=ysl, scalar=1.0, in1=ysl,
                            op0=mybir.AluOpType.mult, op1=mybir.AluOpType.mult,
                            accum_out=sums2[:, c : c + 1],
                        )
                    if gi == 1 and pending is not None:
                        finish_pair(pending)
                        pending = None

                # ---- per-partition stats: totals, cross-partition all-reduce
                # (GpSimd), then mean/E[x^2]/var; the rsqrt half of the chain
                # is deferred into the next pair (finish_pair)
                nc.vector.tensor_reduce(
                    out=st2[:, 0:1], in_=sums[:], axis=mybir.AxisListType.X,
                    op=mybir.AluOpType.add,
                )
                nc.vector.tensor_reduce(
                    out=st2[:, 1:2], in_=sums2[:], axis=mybir.AxisListType.X,
                    op=mybir.AluOpType.add,
                )
                ast = stats.tile([128, 2], f32, tag="ast")
                nc.gpsimd.partition_all_reduce(
                    ast[:], st2[:], 128, bass_isa.ReduceOp.add
                )

                sm = stats.tile([128, 7], f32, tag="sm")
                mean_ex2, mean = sm[:, 0:2], sm[:, 0:1]
                ex2, msq, vpe = sm[:, 1:2], sm[:, 2:3], sm[:, 3:4]
                nc.vector.tensor_scalar_mul(mean_ex2, ast[:], 1.0 / nv)
                nc.vector.tensor_mul(msq, mean, mean)
                nc.vector.tensor_sub(vpe, ex2, msq)

                sb2 = stats.tile([128, 2], f32, tag="sb2")
                o16 = xt[:].rearrange("p a b -> p (a b)")[:, 0:free]
                pending = {"p": p, "sm": sm, "sb2": sb2, "y": y, "o16": o16}
            finish_pair(pending)

    nc.compile()
    return nc


_NC_CACHE = None


def _get_program():
    global _NC_CACHE
    if _NC_CACHE is None:
        _NC_CACHE = build_program()
    return _NC_CACHE


def make_core_inputs(x, w, gamma, beta, core):
    cs = slice(CH_PER_CORE * core, CH_PER_CORE * (core + 1))
    # (b, ci, d, h, w) -> (b, ci, h, d, w) -> (pair, h, d, w), pair = b*8+ci
    xc = np.zeros((N_PAIRS, H, D, WP), np.float16)
    xc[:, :, :, 1 : W + 1] = (
        np.ascontiguousarray(x[:, cs].transpose(0, 1, 3, 2, 4))
        .reshape(N_PAIRS, H, D, W)
        .astype(np.float16)
    )
    bands = np.zeros((CH_PER_CORE, H, 3, 3, H), np.float32)
    eye0 = np.eye(H, dtype=np.float32)
    eyep = np.eye(H, k=1, dtype=np.float32)  # B[h-1, h]: kh=0 tap
    eyem = np.eye(H, k=-1, dtype=np.float32)  # B[h+1, h]: kh=2 tap
    for ci in range(CH_PER_CORE):
        c = CH_PER_CORE * core + ci
        for kd in range(3):
            for kw in range(3):
                wk = w[c, 0, kd, :, kw]
                bands[ci, :, kd, kw, :] = wk[0] * eyep + wk[1] * eye0 + wk[2] * eyem
    gbv = np.broadcast_to(
        np.concatenate([np.tile(gamma[cs], B), np.tile(beta[cs], B)])
        .astype(np.float32)
        .reshape(1, 2 * N_PAIRS),
        (128, 2 * N_PAIRS),
    ).copy()
    bands16 = bands.astype(np.float16)
    d_head = CHUNK_D * 2 + 2
    boot = np.concatenate(
        [bands16[0].reshape(H, 9 * H), xc[0, :, 0:d_head, :].reshape(H, -1)],
        axis=1,
    )
    return {"xs": xc, "bands": bands16, "gb": gbv, "boot": boot}


def kernel(x, w, gamma, beta):
    from concourse.bass_utils import run_bass_kernel_spmd

    x = np.asarray(x, dtype=np.float32)
    w = np.asarray(w, dtype=np.float32)
    gamma = np.asarray(gamma, dtype=np.float32)
    beta = np.asarray(beta, dtype=np.float32)

    nc = _get_program()
    in_maps = [make_core_inputs(x, w, gamma, beta, k) for k in range(N_CORES)]
    res = run_bass_kernel_spmd(nc, in_maps, core_ids=list(range(N_CORES)))

    out = np.empty((B, C, D, H, W), np.float32)
    for k in range(N_CORES):
        cs = slice(CH_PER_CORE * k, CH_PER_CORE * (k + 1))
        yc = (
            res.results[k]["out"]
            .astype(np.float32)
            .reshape(B, CH_PER_CORE, H, D, W)
        )
        out[:, cs] = yc.transpose(0, 1, 3, 2, 4)
    return out

